# revision 6
# baseline (speedup 1.0000x reference)
"""Dense linear layer out = x @ W.T + b on 8 Trainium2 NeuronCores.

Strategy: data-parallel over the batch dim (8192/8 = 1024 rows per core),
W replicated. Host pre-casts both operands to bf16 and lays them out
contraction-major (xt = x_shard.T, wt = W.T) so every DMA is contiguous and
the TensorE contraction dim lands on SBUF partitions. The device kernel is a
tiled matmul: x-shard resident in SBUF (8 MB bf16), W streamed once (32 MB)
as per-n-slab SBUF-cached slabs, fp32 accumulation in PSUM, bias added on
PSUM eviction, fp32 output.

Per-core: M=1024, K=4096, N=4096 -> 2048 matmuls of [128x128]@[128x512].
Each n-slab after the first is processed as two half-groups of 4
output-row blocks so the PSUM evictions of one half hide under the other
half's matmuls.

Schedule notes (from trace analysis; measured 459.1us vs the 443us
zero-stall matmul-stream floor):
- The framework preamble (engine rendezvous) is ~6.6us; no user
  instruction runs before it. The first real matmul additionally cannot
  start before its k-chunks' DMA completion semaphores fire at ~10us —
  each chunk's semaphore lags its last byte by the ~2-3us HBM write
  receipt. 10 dummy warmup matmuls on a memset tile burn that window at
  the HAM-throttled 1.2 GHz clock so the clock gate is open (2.4 GHz) and
  the data is usable right as the real stream starts (~12us). Starting
  the real stream earlier just trades front time for k1-k4 supply stalls
  (the receipt lag applies to every chunk, and startup ring bandwidth is
  only ~140GB/s per ring).
- Startup DMA is the only supply-critical phase: x (8MB) + slab0 (4MB)
  stream k-ordered, alternated across BOTH HWDGE rings (sync + scalar),
  single-k chunks while supply-tight so each k's arrival is its own
  semaphore. Later slabs alternate rings too.
- ns=0 uses the full 8-bank group (minimum per-k demand rate); its
  end-of-slab evictions (425ns vector adds) free banks just ahead of
  ns=1's first group demand.
- The tail: the very last output-row block is computed as two sequential
  256-wide PSUM chains. The first chain's eviction hides under the second
  chain's matmuls, leaving only a 256-wide bias-add + 128KB DMA exposed
  after the final matmul (the remaining ~3.9us is HBM write receipt +
  framework epilogue, which is fixed).
"""

import numpy as np
import ml_dtypes

B, IN, OUT = 8192, 4096, 4096
NCORES = 8
MS = B // NCORES  # 1024 batch rows per core

P = 128
NF = 512            # matmul moving free dim (one PSUM bank of fp32)
KT = IN // P        # 32 contraction tiles
MT = MS // P        # 8 stationary tiles (output partition blocks)
NS = OUT // NF      # 8 output column slabs
HALF = MT // 2      # m-tiles per half-group

SLAB_CHUNK = 4      # k-tiles per wt slab DMA for prefetched slabs
OUT_BUFS = 8

_cache = {}


def _build():
    import concourse.mybir as mybir
    import concourse.tile as tile
    from concourse import bacc

    nc = bacc.Bacc("TRN2", target_bir_lowering=False, debug=False,
                   num_devices=NCORES)
    # row-major contraction-major layouts: the strided per-k DMA patterns
    # (1-2KB contiguous per partition, 8-16KB row strides) measured FASTER
    # than fully SBUF-order-permuted host layouts with 4KB chunks and 64KB
    # partition strides (463us vs 470us) — the fine-grained interleave
    # spreads better across HBM channels
    xt = nc.dram_tensor("xt", [IN, MS], mybir.dt.bfloat16, kind="ExternalInput")
    wt = nc.dram_tensor("wt", [IN, OUT], mybir.dt.bfloat16, kind="ExternalInput")
    bb = nc.dram_tensor("bb", [P, OUT], mybir.dt.float32, kind="ExternalInput")
    out = nc.dram_tensor("out", [MS, OUT], mybir.dt.float32, kind="ExternalOutput")

    xt_t = xt[:].rearrange("(kt p) m -> p kt m", p=P)    # [128, KT, MS]
    wt_t = wt[:].rearrange("(kt p) n -> p kt n", p=P)    # [128, KT, OUT]
    out_t = out[:].rearrange("(mt p) n -> p mt n", p=P)  # [128, MT, OUT]

    with tile.TileContext(nc) as tc:
        with (
            tc.tile_pool(name="xres", bufs=1) as xres_pool,
            tc.tile_pool(name="bias", bufs=1) as bias_pool,
            tc.tile_pool(name="wts", bufs=2) as wts_pool,
            tc.tile_pool(name="psum", bufs=8, space="PSUM") as psum_pool,
            tc.tile_pool(name="outp", bufs=OUT_BUFS) as out_pool,
        ):
            xres = xres_pool.tile([P, KT, MS], mybir.dt.bfloat16)
            bias = bias_pool.tile([P, OUT], mybir.dt.float32)

            # PE warmup: the first real matmul cannot start before its input
            # chunks' DMA completion semaphores fire (~10.5us: framework
            # preamble + issue + HBM write receipt), and the HAM clock gate
            # needs ~3.4us of sustained PE activity to open to 2.4 GHz. Burn
            # that window with dummy matmuls on a memset tile: 10 cold
            # matmuls (~427ns each) bridge from ~7.3us to ~11.6us so the
            # real stream starts warm as its data becomes usable. Starting
            # the real stream earlier than ~12us outruns the chunk supply
            # (per-ring startup bandwidth is only ~140GB/s and every chunk
            # semaphore lags its last byte by the ~3us HBM write receipt).
            wz = bias_pool.tile([P, NF], mybir.dt.bfloat16, name="wz")
            nc.vector.memset(wz[:], 0.0)
            wps = psum_pool.tile([P, NF], mybir.dt.float32,
                                 name="ps", tag="ps")
            for _ in range(10):
                nc.tensor.matmul(wps[:], wz[:, :P], wz[:], start=True,
                                 stop=True)

            # chunk taper: single-k chunks while the stream is supply-tight
            # (each k's arrival is its own completion semaphore), larger
            # later to cut completion round-trips
            chunks = []
            k = 0
            while k < KT:
                step = 1 if k < 4 else (2 if k < 16 else 4)
                chunks.append((k, step))
                k += step

            def prefetch_slab(ns):
                nslc = slice(ns * NF, (ns + 1) * NF)
                slab = wts_pool.tile([P, KT, NF], mybir.dt.bfloat16,
                                     name="wslab", tag="wslab")
                if ns == 0:
                    # startup-critical: x and slab0 interleaved k-ordered and
                    # alternated across both HWDGE rings so neither ring
                    # gates the stream at half bandwidth. The first x chunk
                    # is split so the first matmul's stationary tile (m0)
                    # lands ~200ns earlier.
                    for i, (k, step) in enumerate(chunks):
                        xr = nc.sync if i % 2 == 0 else nc.scalar
                        wr = nc.scalar if i % 2 == 0 else nc.sync
                        if k == 0:
                            xr.dma_start(xres[:, 0:1, 0:256],
                                         xt_t[:, 0:1, 0:256])
                            wr.dma_start(slab[:, 0:1], wt_t[:, 0:1, nslc])
                            xr.dma_start(xres[:, 0:1, 256:MS],
                                         xt_t[:, 0:1, 256:MS])
                        else:
                            xr.dma_start(xres[:, k:k + step],
                                         xt_t[:, k:k + step])
                            wr.dma_start(slab[:, k:k + step],
                                         wt_t[:, k:k + step, nslc])
                else:
                    for i, kc in enumerate(range(0, KT, SLAB_CHUNK)):
                        ring = nc.scalar if i % 2 == 0 else nc.sync
                        ring.dma_start(
                            slab[:, kc:kc + SLAB_CHUNK],
                            wt_t[:, kc:kc + SLAB_CHUNK, nslc])
                return slab

            slab_cur = prefetch_slab(0)
            # bias is first needed by the ns=0 evictions (~55us in); queue it
            # behind the startup-critical phase-1 loads
            nc.scalar.dma_start(bias[:], bb[:])

            for ns in range(NS):
                nslc = slice(ns * NF, (ns + 1) * NF)
                slab_next = prefetch_slab(ns + 1) if ns + 1 < NS else None
                # ns=0 is DMA-supply-limited (x-shard load streams alongside
                # it): the full 8-bank group keeps its per-k DMA demand at
                # the minimum (1.73us per 384KB k-chunk). Its end-of-slab
                # evictions hide under ns=1's first matmuls (each bank is
                # free ~0.8us after its stop-matmul, with 1.5us of cover).
                # Later slabs run from SBUF, so two half-groups let each
                # half's PSUM evictions hide under the other half's matmuls.
                # The last slab tapers; its final m-tile is handled
                # separately below as two sequential 256-wide chains.
                if ns == 0:
                    groups = [range(0, MT)]
                elif ns == NS - 1:
                    groups = [range(0, 4), range(4, 6), range(6, 7)]
                else:
                    groups = [range(h * HALF, (h + 1) * HALF)
                              for h in range(2)]
                for ms in groups:
                    psums = [psum_pool.tile([P, NF], mybir.dt.float32,
                                            name="ps", tag="ps")
                             for _ in ms]
                    for k in range(KT):
                        for i, m in enumerate(ms):
                            nc.tensor.matmul(
                                psums[i][:],
                                xres[:, k, m * P:(m + 1) * P],
                                slab_cur[:, k],
                                start=(k == 0),
                                stop=(k == KT - 1),
                            )
                    for i, m in enumerate(ms):
                        ot = out_pool.tile([P, NF], mybir.dt.float32,
                                           name="ot", tag="ot")
                        nc.vector.tensor_add(ot[:], psums[i][:],
                                             bias[:, nslc])
                        nc.sync.dma_start(out_t[:, m, nslc], ot[:])
                if ns == NS - 1:
                    # trailer m=7: two sequential 256-wide chains in separate
                    # PSUM banks. Chain A's eviction hides under chain B's
                    # matmuls; only chain B's 256-wide add + 128KB DMA are
                    # exposed after the final matmul.
                    m = MT - 1
                    for h in range(2):
                        hsl = slice(h * (NF // 2), (h + 1) * (NF // 2))
                        osl = slice(ns * NF + h * (NF // 2),
                                    ns * NF + (h + 1) * (NF // 2))
                        ps = psum_pool.tile([P, NF], mybir.dt.float32,
                                            name="ps", tag="ps")
                        for k in range(KT):
                            nc.tensor.matmul(
                                ps[:, 0:NF // 2],
                                xres[:, k, m * P:(m + 1) * P],
                                slab_cur[:, k, hsl],
                                start=(k == 0),
                                stop=(k == KT - 1),
                            )
                        ot = out_pool.tile([P, NF // 2], mybir.dt.float32,
                                           name="ot", tag="ot")
                        nc.vector.tensor_add(ot[:], ps[:, 0:NF // 2],
                                             bias[:, osl])
                        nc.sync.dma_start(out_t[:, m, osl], ot[:])
                slab_cur = slab_next

    nc.compile()
    return nc


def prepare_in_maps(x, W, b):
    bf16 = ml_dtypes.bfloat16
    x = np.asarray(x, dtype=np.float32)
    W = np.asarray(W, dtype=np.float32)
    b = np.asarray(b, dtype=np.float32)

    Wt = np.ascontiguousarray(W.astype(bf16).T)                       # [IN, OUT]
    bias = np.ascontiguousarray(
        np.broadcast_to(b.astype(np.float32)[None, :], (P, OUT)))
    xb = x.astype(bf16)

    in_maps = []
    for c in range(NCORES):
        xs = np.ascontiguousarray(xb[c * MS:(c + 1) * MS].T)          # [IN, MS]
        in_maps.append({"xt": xs, "wt": Wt, "bb": bias})
    return in_maps


def kernel(x, W, b):
    from concourse.bass_utils import run_bass_kernel_spmd

    nc = _cache.get("nc")
    if nc is None:
        nc = _cache["nc"] = _build()

    res = run_bass_kernel_spmd(nc, prepare_in_maps(x, W, b),
                               list(range(NCORES)))
    return np.concatenate(
        [res.results[c]["out"] for c in range(NCORES)], axis=0)


# revision 7
# speedup vs baseline: 1.0010x; 1.0010x over previous
"""Dense linear layer out = x @ W.T + b on 8 Trainium2 NeuronCores.

Strategy: data-parallel over the batch dim (8192/8 = 1024 rows per core),
W replicated. Host pre-casts both operands to bf16 and lays them out
contraction-major (xt = x_shard.T, wt = W.T) so every DMA is contiguous and
the TensorE contraction dim lands on SBUF partitions. The device kernel is a
tiled matmul: x-shard resident in SBUF (8 MB bf16), W streamed once (32 MB)
as per-n-slab SBUF-cached slabs, fp32 accumulation in PSUM, bias added on
PSUM eviction, bf16 output upcast to fp32 on host (halves output traffic
and the exposed tail DMA; adds ~2e-3 rel err, still 6x under the 2e-2
gate).

Per-core: M=1024, K=4096, N=4096 -> 2048 matmuls of [128x128]@[128x512].
Each n-slab after the first is processed as two half-groups of 4
output-row blocks so the PSUM evictions of one half hide under the other
half's matmuls.

Schedule notes (from trace analysis; measured 459.1us vs the 443us
zero-stall matmul-stream floor):
- The framework preamble (engine rendezvous) is ~6.6us; no user
  instruction runs before it. The first real matmul additionally cannot
  start before its k-chunks' DMA completion semaphores fire at ~10us —
  each chunk's semaphore lags its last byte by the ~2-3us HBM write
  receipt. 10 dummy warmup matmuls on a memset tile burn that window at
  the HAM-throttled 1.2 GHz clock so the clock gate is open (2.4 GHz) and
  the data is usable right as the real stream starts (~12us). Starting
  the real stream earlier just trades front time for k1-k4 supply stalls
  (the receipt lag applies to every chunk, and startup ring bandwidth is
  only ~140GB/s per ring).
- Startup DMA is the only supply-critical phase: x (8MB) + slab0 (4MB)
  stream k-ordered, alternated across BOTH HWDGE rings (sync + scalar),
  single-k chunks while supply-tight so each k's arrival is its own
  semaphore. Later slabs alternate rings too.
- ns=0 uses the full 8-bank group (minimum per-k demand rate); its
  end-of-slab evictions (425ns vector adds) free banks just ahead of
  ns=1's first group demand.
- The tail: the very last output-row block is computed as two sequential
  256-wide PSUM chains. The first chain's eviction hides under the second
  chain's matmuls, leaving only a 256-wide bias-add + 128KB DMA exposed
  after the final matmul (the remaining ~3.9us is HBM write receipt +
  framework epilogue, which is fixed).
"""

import numpy as np
import ml_dtypes

B, IN, OUT = 8192, 4096, 4096
NCORES = 8
MS = B // NCORES  # 1024 batch rows per core

P = 128
NF = 512            # matmul moving free dim (one PSUM bank of fp32)
KT = IN // P        # 32 contraction tiles
MT = MS // P        # 8 stationary tiles (output partition blocks)
NS = OUT // NF      # 8 output column slabs
HALF = MT // 2      # m-tiles per half-group

SLAB_CHUNK = 4      # k-tiles per wt slab DMA for prefetched slabs
OUT_BUFS = 8

_cache = {}


def _build():
    import concourse.mybir as mybir
    import concourse.tile as tile
    from concourse import bacc

    nc = bacc.Bacc("TRN2", target_bir_lowering=False, debug=False,
                   num_devices=NCORES)
    # row-major contraction-major layouts: the strided per-k DMA patterns
    # (1-2KB contiguous per partition, 8-16KB row strides) measured FASTER
    # than fully SBUF-order-permuted host layouts with 4KB chunks and 64KB
    # partition strides (463us vs 470us) — the fine-grained interleave
    # spreads better across HBM channels
    xt = nc.dram_tensor("xt", [IN, MS], mybir.dt.bfloat16, kind="ExternalInput")
    wt = nc.dram_tensor("wt", [IN, OUT], mybir.dt.bfloat16, kind="ExternalInput")
    bb = nc.dram_tensor("bb", [P, OUT], mybir.dt.float32, kind="ExternalInput")
    out = nc.dram_tensor("out", [MS, OUT], mybir.dt.bfloat16, kind="ExternalOutput")

    xt_t = xt[:].rearrange("(kt p) m -> p kt m", p=P)    # [128, KT, MS]
    wt_t = wt[:].rearrange("(kt p) n -> p kt n", p=P)    # [128, KT, OUT]
    out_t = out[:].rearrange("(mt p) n -> p mt n", p=P)  # [128, MT, OUT]

    with tile.TileContext(nc) as tc:
        with (
            tc.tile_pool(name="xres", bufs=1) as xres_pool,
            tc.tile_pool(name="bias", bufs=1) as bias_pool,
            tc.tile_pool(name="wts", bufs=2) as wts_pool,
            tc.tile_pool(name="psum", bufs=8, space="PSUM") as psum_pool,
            tc.tile_pool(name="outp", bufs=OUT_BUFS) as out_pool,
        ):
            xres = xres_pool.tile([P, KT, MS], mybir.dt.bfloat16)
            bias = bias_pool.tile([P, OUT], mybir.dt.float32)

            # PE warmup: the first real matmul cannot start before its input
            # chunks' DMA completion semaphores fire (~10.5us: framework
            # preamble + issue + HBM write receipt), and the HAM clock gate
            # needs ~3.4us of sustained PE activity to open to 2.4 GHz. Burn
            # that window with dummy matmuls on a memset tile: 10 cold
            # matmuls (~427ns each) bridge from ~7.3us to ~11.6us so the
            # real stream starts warm as its data becomes usable. Starting
            # the real stream earlier than ~12us outruns the chunk supply
            # (per-ring startup bandwidth is only ~140GB/s and every chunk
            # semaphore lags its last byte by the ~3us HBM write receipt).
            wz = bias_pool.tile([P, NF], mybir.dt.bfloat16, name="wz")
            nc.vector.memset(wz[:], 0.0)
            wps = psum_pool.tile([P, NF], mybir.dt.float32,
                                 name="ps", tag="ps")
            for _ in range(10):
                nc.tensor.matmul(wps[:], wz[:, :P], wz[:], start=True,
                                 stop=True)

            # chunk taper: single-k chunks while the stream is supply-tight
            # (each k's arrival is its own completion semaphore), larger
            # later to cut completion round-trips
            chunks = []
            k = 0
            while k < KT:
                step = 1 if k < 4 else (2 if k < 16 else 4)
                chunks.append((k, step))
                k += step

            def prefetch_slab(ns):
                nslc = slice(ns * NF, (ns + 1) * NF)
                slab = wts_pool.tile([P, KT, NF], mybir.dt.bfloat16,
                                     name="wslab", tag="wslab")
                if ns == 0:
                    # startup-critical: x and slab0 interleaved k-ordered and
                    # alternated across both HWDGE rings so neither ring
                    # gates the stream at half bandwidth. The first x chunk
                    # is split so the first matmul's stationary tile (m0)
                    # lands ~200ns earlier.
                    for i, (k, step) in enumerate(chunks):
                        xr = nc.sync if i % 2 == 0 else nc.scalar
                        wr = nc.scalar if i % 2 == 0 else nc.sync
                        if k == 0:
                            xr.dma_start(xres[:, 0:1, 0:256],
                                         xt_t[:, 0:1, 0:256])
                            wr.dma_start(slab[:, 0:1], wt_t[:, 0:1, nslc])
                            xr.dma_start(xres[:, 0:1, 256:MS],
                                         xt_t[:, 0:1, 256:MS])
                        else:
                            xr.dma_start(xres[:, k:k + step],
                                         xt_t[:, k:k + step])
                            wr.dma_start(slab[:, k:k + step],
                                         wt_t[:, k:k + step, nslc])
                else:
                    for i, kc in enumerate(range(0, KT, SLAB_CHUNK)):
                        ring = nc.scalar if i % 2 == 0 else nc.sync
                        ring.dma_start(
                            slab[:, kc:kc + SLAB_CHUNK],
                            wt_t[:, kc:kc + SLAB_CHUNK, nslc])
                return slab

            slab_cur = prefetch_slab(0)
            # bias is first needed by the ns=0 evictions (~55us in); queue it
            # behind the startup-critical phase-1 loads
            nc.scalar.dma_start(bias[:], bb[:])

            for ns in range(NS):
                nslc = slice(ns * NF, (ns + 1) * NF)
                slab_next = prefetch_slab(ns + 1) if ns + 1 < NS else None
                # ns=0 is DMA-supply-limited (x-shard load streams alongside
                # it): the full 8-bank group keeps its per-k DMA demand at
                # the minimum (1.73us per 384KB k-chunk). Its end-of-slab
                # evictions hide under ns=1's first matmuls (each bank is
                # free ~0.8us after its stop-matmul, with 1.5us of cover).
                # Later slabs run from SBUF, so two half-groups let each
                # half's PSUM evictions hide under the other half's matmuls.
                # The last slab tapers; its final m-tile is handled
                # separately below as two sequential 256-wide chains.
                if ns == 0:
                    groups = [range(0, MT)]
                elif ns == NS - 1:
                    groups = [range(0, 4), range(4, 6), range(6, 7)]
                else:
                    groups = [range(h * HALF, (h + 1) * HALF)
                              for h in range(2)]
                for ms in groups:
                    psums = [psum_pool.tile([P, NF], mybir.dt.float32,
                                            name="ps", tag="ps")
                             for _ in ms]
                    for k in range(KT):
                        for i, m in enumerate(ms):
                            nc.tensor.matmul(
                                psums[i][:],
                                xres[:, k, m * P:(m + 1) * P],
                                slab_cur[:, k],
                                start=(k == 0),
                                stop=(k == KT - 1),
                            )
                    for i, m in enumerate(ms):
                        ot = out_pool.tile([P, NF], mybir.dt.bfloat16,
                                           name="ot", tag="ot")
                        nc.vector.tensor_add(ot[:], psums[i][:],
                                             bias[:, nslc])
                        nc.sync.dma_start(out_t[:, m, nslc], ot[:])
                if ns == NS - 1:
                    # trailer m=7: two sequential 256-wide chains in separate
                    # PSUM banks. Chain A's eviction hides under chain B's
                    # matmuls; only chain B's 256-wide add + 128KB DMA are
                    # exposed after the final matmul.
                    m = MT - 1
                    for h in range(2):
                        hsl = slice(h * (NF // 2), (h + 1) * (NF // 2))
                        osl = slice(ns * NF + h * (NF // 2),
                                    ns * NF + (h + 1) * (NF // 2))
                        ps = psum_pool.tile([P, NF], mybir.dt.float32,
                                            name="ps", tag="ps")
                        for k in range(KT):
                            nc.tensor.matmul(
                                ps[:, 0:NF // 2],
                                xres[:, k, m * P:(m + 1) * P],
                                slab_cur[:, k, hsl],
                                start=(k == 0),
                                stop=(k == KT - 1),
                            )
                        ot = out_pool.tile([P, NF // 2], mybir.dt.bfloat16,
                                           name="ot", tag="ot")
                        nc.vector.tensor_add(ot[:], ps[:, 0:NF // 2],
                                             bias[:, osl])
                        nc.sync.dma_start(out_t[:, m, osl], ot[:])
                slab_cur = slab_next

    nc.compile()
    return nc


def prepare_in_maps(x, W, b):
    bf16 = ml_dtypes.bfloat16
    x = np.asarray(x, dtype=np.float32)
    W = np.asarray(W, dtype=np.float32)
    b = np.asarray(b, dtype=np.float32)

    Wt = np.ascontiguousarray(W.astype(bf16).T)                       # [IN, OUT]
    bias = np.ascontiguousarray(
        np.broadcast_to(b.astype(np.float32)[None, :], (P, OUT)))
    xb = x.astype(bf16)

    in_maps = []
    for c in range(NCORES):
        xs = np.ascontiguousarray(xb[c * MS:(c + 1) * MS].T)          # [IN, MS]
        in_maps.append({"xt": xs, "wt": Wt, "bb": bias})
    return in_maps


def kernel(x, W, b):
    from concourse.bass_utils import run_bass_kernel_spmd

    nc = _cache.get("nc")
    if nc is None:
        nc = _cache["nc"] = _build()

    res = run_bass_kernel_spmd(nc, prepare_in_maps(x, W, b),
                               list(range(NCORES)))
    return np.concatenate(
        [res.results[c]["out"] for c in range(NCORES)],
        axis=0).astype(np.float32)
